# revision 5
# baseline (speedup 1.0000x reference)
"""Trainium2 Bass kernel for nn_ChebychevInput.

out[b,o,s] = sum_{i,p} (WEIGHT_MAGNITUDE*coef[o,i,p]) * cos(p*arccos(x[b,i,s]))

Device pipeline per core (s-shard of 16384, both batches):
  theta-stage (tiny, flat [96,1024] layout):
      a = arctan(x/sqrt(1-x^2)) = arcsin(x);  theta = pi/2 - a
      theta' = theta * 2^16/(2pi)   (cycles in 2^16 units)
  per (b, s-chunk):
      GPSIMD partition_broadcast -> th3[126, SC] (42 rows per i)
      DVE   (x7 k-tiles): Y32 = int32(th3 * p + 0.25*2^16)   [one pass]
      ACT   one Sin over the int16-bitcast low halfwords: T = sin(2pi*Y/2^16)
            = cos(2pi * p*theta/(2pi)) = cos(p*theta)   -> fp16
      PE    out[o,s] accumulated over 7 k-tiles: lhsT = W[126,128] fp16
      DVE   PSUM -> SBUF fp16, DMA -> out (fp16 halves the device->host bytes)
Row packing: k-tile kt row j: i = j//42, p = 42*kt + j%42  (k=126 rows/tile).

Host side: per-device jitted programs (no shard_map mesh), cached across
calls; zero output buffers are created on-device and donated; the 8 fp16
output shards are fetched over the tunnel in parallel threads and
upcast/assembled into the full fp32 array on the host.
"""
import sys

sys.path.insert(0, "/opt/trn_rl_repo")

from concurrent.futures import ThreadPoolExecutor

import numpy as np

BATCH = 2
INPUT_DIM = 3
N_SAMPLES = 131072
OUTPUT_DIM = 256
POLY_DEGREE = 256  # p = 0..256 -> 257 values
N_CORES = 8
S_SHARD = N_SAMPLES // N_CORES  # 16384
SC = 1024                       # sample chunk
NSC = S_SHARD // SC             # 16
NKT = 7                         # k-tiles of 126 rows (3i x 42p)
KT_ROWS = 126
WEIGHT_MAGNITUDE = float(np.sqrt(6.0 / (INPUT_DIM * (POLY_DEGREE + 1))))
TWO16 = 65536.0

_compiled = {}
_pool = ThreadPoolExecutor(N_CORES)


def _build():
    import concourse.tile as tile
    from concourse import bacc, mybir

    F32 = mybir.dt.float32
    F16 = mybir.dt.float16
    I32 = mybir.dt.int32
    I16 = mybir.dt.int16
    AF = mybir.ActivationFunctionType
    ALU = mybir.AluOpType

    nc = bacc.Bacc("TRN2", target_bir_lowering=False, debug=False)
    x_d = nc.dram_tensor("x", [BATCH, INPUT_DIM, S_SHARD], F32, kind="ExternalInput")
    w_d = nc.dram_tensor("w", [KT_ROWS, NKT * OUTPUT_DIM], F16, kind="ExternalInput")
    pc_d = nc.dram_tensor("pc", [KT_ROWS, NKT], F32, kind="ExternalInput")
    out_d = nc.dram_tensor("out", [BATCH, OUTPUT_DIM, S_SHARD], F16, kind="ExternalOutput")

    with tile.TileContext(nc) as tc:
        with (
            tc.tile_pool(name="const", bufs=1) as constp,
            tc.tile_pool(name="theta", bufs=1) as thp,
            tc.tile_pool(name="bcast", bufs=2) as bcp,
            tc.tile_pool(name="yint", bufs=2) as yp,
            tc.tile_pool(name="tmat", bufs=2) as tp,
            tc.tile_pool(name="outs", bufs=4) as op,
            tc.tile_pool(name="psum", bufs=4, space="PSUM") as pp,
        ):
            w_t = constp.tile([KT_ROWS, NKT * OUTPUT_DIM], F16)
            nc.sync.dma_start(w_t[:], w_d[:])
            pc_t = constp.tile([KT_ROWS, NKT], F32)
            nc.sync.dma_start(pc_t[:], pc_d[:])

            # ---- theta stage: flat [96, 1024]; row = 48*b + 16*i + u, u = s-chunk
            xt = thp.tile([96, 1024], F32)
            nc.sync.dma_start(xt[:], x_d[:].rearrange("b i (u c) -> (b i u) c", c=1024))
            sq = thp.tile([96, 1024], F32)
            nc.scalar.activation(sq[:], xt[:], AF.Square)
            r2 = thp.tile([96, 1024], F32)
            nc.scalar.activation(r2[:], sq[:], AF.Sqrt, bias=1.0, scale=-1.0)
            inv = thp.tile([96, 1024], F32)
            nc.vector.reciprocal(inv[:], r2[:])
            q = thp.tile([96, 1024], F32)
            nc.vector.tensor_mul(q[:], xt[:], inv[:])
            asn = thp.tile([96, 1024], F32)
            nc.scalar.activation(asn[:], q[:], AF.Arctan)
            # theta' = (pi/2 - a) * 2^16/(2pi) = 2^14 - a * (2^16/2pi)
            thf = thp.tile([96, 1024], F32)
            nc.scalar.activation(thf[:], asn[:], AF.Copy,
                                 bias=16384.0, scale=float(-TWO16 / (2 * np.pi)))

            # ---- main loops
            for b in range(BATCH):
                for sc in range(NSC):
                    th3 = bcp.tile([KT_ROWS, SC], F32)
                    for i in range(INPUT_DIM):
                        row = 48 * b + 16 * i + sc
                        tmp = bcp.tile([1, SC], F32, tag=f"throw{i}")
                        nc.sync.dma_start(tmp[:], thf[row:row + 1, :])
                        bc = bcp.tile([42, SC], F32, tag=f"thbc{i}")
                        nc.gpsimd.partition_broadcast(bc[:], tmp[:])
                        nc.sync.dma_start(th3[42 * i:42 * (i + 1), :], bc[:])
                    y32 = yp.tile([KT_ROWS, NKT * SC], I32)
                    for kt in range(NKT):
                        nc.vector.tensor_scalar(
                            y32[:, kt * SC:(kt + 1) * SC], th3[:],
                            pc_t[:, kt:kt + 1], 0.25 * TWO16, ALU.mult, ALU.add,
                        )
                    tm = tp.tile([KT_ROWS, NKT * SC], F16)
                    yv = y32[:].bitcast(I16).rearrange("p (n two) -> p n two", two=2)[:, :, 0]
                    nc.scalar.activation(tm[:], yv, AF.Sin, scale=float(2 * np.pi / TWO16))

                    for m in range(2):
                        for half in range(2):
                            ps = pp.tile([128, 512], F32)
                            for kt in range(NKT):
                                nc.tensor.matmul(
                                    ps[:],
                                    w_t[:, kt * OUTPUT_DIM + m * 128: kt * OUTPUT_DIM + m * 128 + 128],
                                    tm[:, kt * SC + half * 512: kt * SC + half * 512 + 512],
                                    start=(kt == 0), stop=(kt == NKT - 1),
                                )
                            ob = op.tile([128, 512], F16)
                            nc.vector.tensor_copy(ob[:], ps[:])
                            nc.sync.dma_start(
                                out_d[b, m * 128:(m + 1) * 128,
                                      sc * SC + half * 512: sc * SC + half * 512 + 512],
                                ob[:],
                            )
    nc.compile()
    return nc


def _host_prep(coefficients):
    w = (coefficients.astype(np.float64) * WEIGHT_MAGNITUDE).astype(np.float32)
    # w: (256, 3, 257) -> lhsT rows j (i=j//42, p=42*kt+j%42), cols kt*256+o
    wk = np.zeros((KT_ROWS, NKT * OUTPUT_DIM), np.float32)
    j = np.arange(KT_ROWS)
    ii = j // 42
    for kt in range(NKT):
        pp_ = 42 * kt + (j % 42)
        valid = pp_ <= POLY_DEGREE
        # wk[j, kt*256 + o] = w[o, ii[j], pp_[j]]
        wk[valid, kt * OUTPUT_DIM:(kt + 1) * OUTPUT_DIM] = \
            w[:, ii[valid], pp_[valid]].T
    pc = np.zeros((KT_ROWS, NKT), np.float32)
    for kt in range(NKT):
        pc[:, kt] = 42 * kt + (j % 42)
    return wk.astype(np.float16), pc


def _get_nc():
    if "nc" not in _compiled:
        _compiled["nc"] = _build()
    return _compiled["nc"]


def _get_exec_fn():
    """Single-device jitted bass exec: (x, w, pc, zero_out) -> out (donated)."""
    if "exec_fn" in _compiled:
        return _compiled["exec_fn"]
    import jax
    from concourse.bass2jax import (
        _bass_exec_p, install_neuronx_cc_hook, partition_id_tensor)

    nc = _get_nc()
    install_neuronx_cc_hook()
    out_aval = jax.core.ShapedArray((BATCH, OUTPUT_DIM, S_SHARD), np.float16)
    pname = nc.partition_id_tensor.name if nc.partition_id_tensor else None
    in_names = ["x", "w", "pc", "out"]
    if pname is not None:
        in_names.append(pname)

    def _body(xs, ws, pcs, zs):
        operands = [xs, ws, pcs, zs]
        if pname is not None:
            operands.append(partition_id_tensor())
        (out,) = _bass_exec_p.bind(
            *operands,
            out_avals=(out_aval,),
            in_names=tuple(in_names),
            out_names=("out",),
            lowering_input_output_aliases=(),
            sim_require_finite=True,
            sim_require_nnan=True,
            nc=nc,
        )
        return out

    fn = jax.jit(_body, donate_argnums=(3,), keep_unused=True)
    _compiled["exec_fn"] = fn
    return fn


def _get_zeros_fns():
    """Per-device jitted on-device zero-buffer factories (nothing crosses
    the tunnel)."""
    if "zeros_fns" in _compiled:
        return _compiled["zeros_fns"]
    import jax
    import jax.numpy as jnp
    from jax.sharding import SingleDeviceSharding

    fns = []
    for d in jax.devices()[:N_CORES]:
        fns.append(jax.jit(
            lambda: jnp.zeros((BATCH, OUTPUT_DIM, S_SHARD), jnp.float16),
            out_shardings=SingleDeviceSharding(d)))
    _compiled["zeros_fns"] = fns
    return fns


def _get_weights_on_dev(coefficients):
    """Device-resident weight/pc arrays, cached across calls by content."""
    import jax

    key = ("wdev", hash(coefficients.tobytes()))
    if key in _compiled:
        return _compiled[key]
    wk, pc = _host_prep(np.asarray(coefficients, dtype=np.float32))
    devs = jax.devices()[:N_CORES]
    wds = [jax.device_put(wk, d) for d in devs]
    pcds = [jax.device_put(pc, d) for d in devs]
    _compiled[key] = (wds, pcds)
    return _compiled[key]


def _launch(x_np, wds, pcds):
    """Shard x, ship to devices, run the bass program on all 8 cores.

    Returns the list of device-resident fp16 output shards (async)."""
    import jax

    devs = jax.devices()[:N_CORES]
    fn = _get_exec_fn()
    zfns = _get_zeros_fns()
    xds = [
        jax.device_put(
            np.ascontiguousarray(x_np[:, :, c * S_SHARD:(c + 1) * S_SHARD]), devs[c])
        for c in range(N_CORES)
    ]
    outs = []
    for c in range(N_CORES):
        z = zfns[c]()
        outs.append(fn(xds[c], wds[c], pcds[c], z))
    return outs


def _fetch_assemble(outs):
    """Parallel d2h of the 8 fp16 shards + threaded upcast into full fp32."""
    for o in outs:
        o.copy_to_host_async()
    res = np.empty((BATCH, OUTPUT_DIM, N_SAMPLES), np.float32)

    def grab(c):
        shard = np.asarray(outs[c])  # (2, 256, 16384) fp16
        res[:, :, c * S_SHARD:(c + 1) * S_SHARD] = shard
        return None

    list(_pool.map(grab, range(N_CORES)))
    return res


def kernel(x, coefficients):
    x = np.asarray(x, dtype=np.float32)
    coefficients = np.asarray(coefficients, dtype=np.float32)
    wds, pcds = _get_weights_on_dev(coefficients)
    outs = _launch(x, wds, pcds)
    return _fetch_assemble(outs)


# ---------------------------------------------------------------------------
# helpers kept for test.py's differential timing path
# ---------------------------------------------------------------------------

def _prep_globals(x, coefficients):
    wk, pc = _host_prep(np.asarray(coefficients, dtype=np.float32))
    xg = np.ascontiguousarray(
        np.asarray(x, dtype=np.float32).reshape(BATCH, INPUT_DIM, N_CORES, S_SHARD)
        .transpose(2, 0, 1, 3).reshape(N_CORES * BATCH, INPUT_DIM, S_SHARD))
    wg = np.tile(wk, (N_CORES, 1))
    pcg = np.tile(pc, (N_CORES, 1))
    return xg, wg, pcg


class _ZeroList(list):
    def block_until_ready(self):
        import jax

        jax.block_until_ready(list(self))
        return self


def _make_zeros():
    """Fresh on-device zero output buffers, one per core (donated per exec)."""
    zfns = _get_zeros_fns()
    return _ZeroList(zf() for zf in zfns)


def _get_callable(n_execs=1):
    """Callable running n_execs chained bass execs per core on 8 cores.

    Signature matches test.py: f(xg, wg, pcg, zeros_list) -> object with
    .block_until_ready(). Weights/x are device-resident jax arrays or np
    arrays; each exec donates the previous output (same shape/dtype), so
    t(2 execs) - t(1 exec) isolates one on-device execution.
    """
    import jax

    key = ("fn", n_execs)
    if key in _compiled:
        return _compiled[key]

    fn = _get_exec_fn()
    devs_n = N_CORES
    devput_cache = _compiled.setdefault("devput_cache", {})

    def run(xg, wg, pcg, zs):
        import jax as _jax

        ck = (id(xg), id(wg), id(pcg))
        if ck not in devput_cache:
            devs = _jax.devices()[:devs_n]
            xs = [np.asarray(xg[c * BATCH:(c + 1) * BATCH]) for c in range(devs_n)]
            ws = [np.asarray(wg[c * KT_ROWS:(c + 1) * KT_ROWS]) for c in range(devs_n)]
            pcs = [np.asarray(pcg[c * KT_ROWS:(c + 1) * KT_ROWS]) for c in range(devs_n)]
            xs = [_jax.device_put(a, d) for a, d in zip(xs, devs)]
            ws = [_jax.device_put(a, d) for a, d in zip(ws, devs)]
            pcs = [_jax.device_put(a, d) for a, d in zip(pcs, devs)]
            _jax.block_until_ready(xs + ws + pcs)
            devput_cache[ck] = (xs, ws, pcs)
        xs, ws, pcs = devput_cache[ck]
        outs = list(zs)
        for _ in range(n_execs):
            outs = [fn(xs[c], ws[c], pcs[c], outs[c]) for c in range(devs_n)]

        class _R:
            def __init__(self, arrs):
                self.arrs = arrs

            def block_until_ready(self):
                _jax.block_until_ready(self.arrs)
                return self

        return _R(outs)

    _compiled[key] = run
    return run


# revision 6
# speedup vs baseline: 8.8533x; 8.8533x over previous
"""Trainium2 Bass kernel for nn_ChebychevInput.

out[b,o,s] = sum_{i,p} (WEIGHT_MAGNITUDE*coef[o,i,p]) * cos(p*arccos(x[b,i,s]))

Device pipeline per core (s-shard of 16384, both batches):
  theta-stage (tiny, flat [96,1024] layout):
      a = arctan(x/sqrt(1-x^2)) = arcsin(x);  theta = pi/2 - a
      theta' = theta * 2^16/(2pi)   (cycles in 2^16 units)
  per (b, s-chunk):
      GPSIMD partition_broadcast -> th3[126, SC] (42 rows per i)
      DVE   (x7 k-tiles): Y32 = int32(th3 * p + 0.25*2^16)   [one pass]
      ACT   one Sin over the int16-bitcast low halfwords: T = sin(2pi*Y/2^16)
            = cos(2pi * p*theta/(2pi)) = cos(p*theta)   -> fp16
      PE    out[o,s] accumulated over 7 k-tiles: lhsT = W[126,128] fp16
      DVE   PSUM -> SBUF fp16, DMA -> out (fp16 halves the device->host bytes)
Row packing: k-tile kt row j: i = j//42, p = 42*kt + j%42  (k=126 rows/tile).

Host side: ONE cached jit(shard_map) exec program over the 8-core mesh and
ONE cached sharded-zeros program (donated output buffers); weights are
device-resident across calls; the 8 fp16 output shards are fetched over
the tunnel in parallel threads and upcast into the full fp32 array.
"""
import sys

sys.path.insert(0, "/opt/trn_rl_repo")

from concurrent.futures import ThreadPoolExecutor

import numpy as np

BATCH = 2
INPUT_DIM = 3
N_SAMPLES = 131072
OUTPUT_DIM = 256
POLY_DEGREE = 256  # p = 0..256 -> 257 values
N_CORES = 8
S_SHARD = N_SAMPLES // N_CORES  # 16384
SC = 1024                       # sample chunk
NSC = S_SHARD // SC             # 16
NKT = 7                         # k-tiles of 126 rows (3i x 42p)
KT_ROWS = 126
WEIGHT_MAGNITUDE = float(np.sqrt(6.0 / (INPUT_DIM * (POLY_DEGREE + 1))))
TWO16 = 65536.0

_compiled = {}
_pool = ThreadPoolExecutor(N_CORES)


def _build():
    import concourse.tile as tile
    from concourse import bacc, mybir

    F32 = mybir.dt.float32
    F16 = mybir.dt.float16
    I32 = mybir.dt.int32
    I16 = mybir.dt.int16
    AF = mybir.ActivationFunctionType
    ALU = mybir.AluOpType

    nc = bacc.Bacc("TRN2", target_bir_lowering=False, debug=False)
    x_d = nc.dram_tensor("x", [BATCH, INPUT_DIM, S_SHARD], F32, kind="ExternalInput")
    w_d = nc.dram_tensor("w", [KT_ROWS, NKT * OUTPUT_DIM], F16, kind="ExternalInput")
    pc_d = nc.dram_tensor("pc", [KT_ROWS, NKT], F32, kind="ExternalInput")
    out_d = nc.dram_tensor("out", [BATCH, OUTPUT_DIM, S_SHARD], F16, kind="ExternalOutput")

    with tile.TileContext(nc) as tc:
        with (
            tc.tile_pool(name="const", bufs=1) as constp,
            tc.tile_pool(name="theta", bufs=1) as thp,
            tc.tile_pool(name="bcast", bufs=2) as bcp,
            tc.tile_pool(name="yint", bufs=2) as yp,
            tc.tile_pool(name="tmat", bufs=2) as tp,
            tc.tile_pool(name="outs", bufs=4) as op,
            tc.tile_pool(name="psum", bufs=4, space="PSUM") as pp,
        ):
            w_t = constp.tile([KT_ROWS, NKT * OUTPUT_DIM], F16)
            nc.sync.dma_start(w_t[:], w_d[:])
            pc_t = constp.tile([KT_ROWS, NKT], F32)
            nc.sync.dma_start(pc_t[:], pc_d[:])

            # ---- theta stage: flat [96, 1024]; row = 48*b + 16*i + u, u = s-chunk
            xt = thp.tile([96, 1024], F32)
            nc.sync.dma_start(xt[:], x_d[:].rearrange("b i (u c) -> (b i u) c", c=1024))
            sq = thp.tile([96, 1024], F32)
            nc.scalar.activation(sq[:], xt[:], AF.Square)
            r2 = thp.tile([96, 1024], F32)
            nc.scalar.activation(r2[:], sq[:], AF.Sqrt, bias=1.0, scale=-1.0)
            inv = thp.tile([96, 1024], F32)
            nc.vector.reciprocal(inv[:], r2[:])
            q = thp.tile([96, 1024], F32)
            nc.vector.tensor_mul(q[:], xt[:], inv[:])
            asn = thp.tile([96, 1024], F32)
            nc.scalar.activation(asn[:], q[:], AF.Arctan)
            # theta' = (pi/2 - a) * 2^16/(2pi) = 2^14 - a * (2^16/2pi)
            thf = thp.tile([96, 1024], F32)
            nc.scalar.activation(thf[:], asn[:], AF.Copy,
                                 bias=16384.0, scale=float(-TWO16 / (2 * np.pi)))

            # ---- main loops
            for b in range(BATCH):
                for sc in range(NSC):
                    th3 = bcp.tile([KT_ROWS, SC], F32)
                    for i in range(INPUT_DIM):
                        row = 48 * b + 16 * i + sc
                        tmp = bcp.tile([1, SC], F32, tag=f"throw{i}")
                        nc.sync.dma_start(tmp[:], thf[row:row + 1, :])
                        bc = bcp.tile([42, SC], F32, tag=f"thbc{i}")
                        nc.gpsimd.partition_broadcast(bc[:], tmp[:])
                        nc.sync.dma_start(th3[42 * i:42 * (i + 1), :], bc[:])
                    y32 = yp.tile([KT_ROWS, NKT * SC], I32)
                    for kt in range(NKT):
                        nc.vector.tensor_scalar(
                            y32[:, kt * SC:(kt + 1) * SC], th3[:],
                            pc_t[:, kt:kt + 1], 0.25 * TWO16, ALU.mult, ALU.add,
                        )
                    tm = tp.tile([KT_ROWS, NKT * SC], F16)
                    yv = y32[:].bitcast(I16).rearrange("p (n two) -> p n two", two=2)[:, :, 0]
                    nc.scalar.activation(tm[:], yv, AF.Sin, scale=float(2 * np.pi / TWO16))

                    for m in range(2):
                        for half in range(2):
                            ps = pp.tile([128, 512], F32)
                            for kt in range(NKT):
                                nc.tensor.matmul(
                                    ps[:],
                                    w_t[:, kt * OUTPUT_DIM + m * 128: kt * OUTPUT_DIM + m * 128 + 128],
                                    tm[:, kt * SC + half * 512: kt * SC + half * 512 + 512],
                                    start=(kt == 0), stop=(kt == NKT - 1),
                                )
                            ob = op.tile([128, 512], F16)
                            nc.vector.tensor_copy(ob[:], ps[:])
                            nc.sync.dma_start(
                                out_d[b, m * 128:(m + 1) * 128,
                                      sc * SC + half * 512: sc * SC + half * 512 + 512],
                                ob[:],
                            )
    nc.compile()
    return nc


def _host_prep(coefficients):
    w = (coefficients.astype(np.float64) * WEIGHT_MAGNITUDE).astype(np.float32)
    # w: (256, 3, 257) -> lhsT rows j (i=j//42, p=42*kt+j%42), cols kt*256+o
    wk = np.zeros((KT_ROWS, NKT * OUTPUT_DIM), np.float32)
    j = np.arange(KT_ROWS)
    ii = j // 42
    for kt in range(NKT):
        pp_ = 42 * kt + (j % 42)
        valid = pp_ <= POLY_DEGREE
        # wk[j, kt*256 + o] = w[o, ii[j], pp_[j]]
        wk[valid, kt * OUTPUT_DIM:(kt + 1) * OUTPUT_DIM] = \
            w[:, ii[valid], pp_[valid]].T
    pc = np.zeros((KT_ROWS, NKT), np.float32)
    for kt in range(NKT):
        pc[:, kt] = 42 * kt + (j % 42)
    return wk.astype(np.float16), pc


def _get_nc():
    if "nc" not in _compiled:
        _compiled["nc"] = _build()
    return _compiled["nc"]


def _get_mesh():
    if "mesh" not in _compiled:
        import jax
        from jax.sharding import Mesh

        _compiled["mesh"] = Mesh(np.asarray(jax.devices()[:N_CORES]), ("core",))
    return _compiled["mesh"]


def _core_sharding():
    if "shard" not in _compiled:
        from jax.sharding import NamedSharding, PartitionSpec

        _compiled["shard"] = NamedSharding(_get_mesh(), PartitionSpec("core"))
    return _compiled["shard"]


def _get_exec_fn():
    """Cached jit(shard_map) bass exec over the 8-core mesh.

    (x[16,3,16384], w[8*126,1792], pc[8*126,7], zeros[16,256,16384]f16)
      -> out[16,256,16384]f16, zeros donated.
    """
    if "exec_fn" in _compiled:
        return _compiled["exec_fn"]
    import jax
    from jax.experimental.shard_map import shard_map
    from jax.sharding import PartitionSpec
    from concourse.bass2jax import (
        _bass_exec_p, install_neuronx_cc_hook, partition_id_tensor)

    nc = _get_nc()
    install_neuronx_cc_hook()
    out_aval = jax.core.ShapedArray((BATCH, OUTPUT_DIM, S_SHARD), np.float16)
    pname = nc.partition_id_tensor.name if nc.partition_id_tensor else None
    in_names = ["x", "w", "pc", "out"]
    if pname is not None:
        in_names.append(pname)

    def _body(xs, ws, pcs, zs):
        operands = [xs, ws, pcs, zs]
        if pname is not None:
            operands.append(partition_id_tensor())
        (out,) = _bass_exec_p.bind(
            *operands,
            out_avals=(out_aval,),
            in_names=tuple(in_names),
            out_names=("out",),
            lowering_input_output_aliases=(),
            sim_require_finite=True,
            sim_require_nnan=True,
            nc=nc,
        )
        return out

    mesh = _get_mesh()
    fn = jax.jit(
        shard_map(
            _body, mesh=mesh,
            in_specs=(PartitionSpec("core"),) * 4,
            out_specs=PartitionSpec("core"),
            check_rep=False,
        ),
        donate_argnums=(3,), keep_unused=True)
    _compiled["exec_fn"] = fn
    return fn


def _get_zeros_fn():
    """Cached on-device sharded zero-output factory (nothing crosses the
    tunnel)."""
    if "zeros_fn" in _compiled:
        return _compiled["zeros_fn"]
    import jax
    import jax.numpy as jnp

    fn = jax.jit(
        lambda: jnp.zeros((N_CORES * BATCH, OUTPUT_DIM, S_SHARD), jnp.float16),
        out_shardings=_core_sharding())
    _compiled["zeros_fn"] = fn
    return fn


def _donate_buf():
    """Buffer to donate as the exec output: last call's output if alive,
    else fresh on-device zeros (our kernel writes every output element)."""
    buf = _compiled.pop("scratch", None)
    if buf is not None:
        return buf
    return _get_zeros_fn()()


def _get_weights_on_dev(coefficients):
    """Device-resident sharded weight/pc globals, cached by content."""
    import jax

    key = ("wdev", hash(coefficients.tobytes()))
    if key in _compiled:
        return _compiled[key]
    wk, pc = _host_prep(np.asarray(coefficients, dtype=np.float32))
    sh = _core_sharding()
    wg = jax.device_put(np.tile(wk, (N_CORES, 1)), sh)
    pcg = jax.device_put(np.tile(pc, (N_CORES, 1)), sh)
    jax.block_until_ready([wg, pcg])
    _compiled[key] = (wg, pcg)
    return _compiled[key]


def _shard_x(x_np):
    return np.ascontiguousarray(
        x_np.reshape(BATCH, INPUT_DIM, N_CORES, S_SHARD)
        .transpose(2, 0, 1, 3).reshape(N_CORES * BATCH, INPUT_DIM, S_SHARD))


def _fetch_assemble(out_g):
    """Parallel d2h of the 8 fp16 shards + threaded upcast into full fp32."""
    import jax

    dev_to_core = {d: c for c, d in enumerate(jax.devices()[:N_CORES])}
    shards = list(out_g.addressable_shards)
    res = np.empty((BATCH, OUTPUT_DIM, N_SAMPLES), np.float32)

    def grab(sh):
        c = dev_to_core[sh.device]
        arr = np.asarray(sh.data)  # (2, 256, 16384) fp16
        res[:, :, c * S_SHARD:(c + 1) * S_SHARD] = arr
        return None

    list(_pool.map(grab, shards))
    return res


def kernel(x, coefficients):
    import jax

    x = np.asarray(x, dtype=np.float32)
    coefficients = np.asarray(coefficients, dtype=np.float32)
    wg, pcg = _get_weights_on_dev(coefficients)
    fn = _get_exec_fn()
    xg = jax.device_put(_shard_x(x), _core_sharding())
    out_g = fn(xg, wg, pcg, _donate_buf())
    res = _fetch_assemble(out_g)
    _compiled["scratch"] = out_g  # donate next call
    return res


# ---------------------------------------------------------------------------
# helpers kept for test.py's differential timing path
# ---------------------------------------------------------------------------

def _prep_globals(x, coefficients):
    wk, pc = _host_prep(np.asarray(coefficients, dtype=np.float32))
    xg = _shard_x(np.asarray(x, dtype=np.float32))
    wg = np.tile(wk, (N_CORES, 1))
    pcg = np.tile(pc, (N_CORES, 1))
    return xg, wg, pcg


def _make_zeros():
    """Fresh on-device sharded zero output buffer (donated into each exec)."""
    return _get_zeros_fn()()


def _get_callable(n_execs=1):
    """Callable running n_execs chained bass execs on the 8-core mesh.

    f(xg, wg, pcg, zeros) -> sharded out array; each exec donates the
    previous buffer, so t(2 execs) - t(1 exec) isolates one on-device
    execution round. Inputs may be np arrays or jax arrays; device
    placement (with the mesh sharding) is cached by id across calls.
    """
    key = ("fn", n_execs)
    if key in _compiled:
        return _compiled[key]

    fn = _get_exec_fn()
    devput_cache = _compiled.setdefault("devput_cache", {})

    def run(xg, wg, pcg, zs):
        import jax

        ck = (id(xg), id(wg), id(pcg))
        if ck not in devput_cache:
            sh = _core_sharding()
            placed = [jax.device_put(np.asarray(a), sh) for a in (xg, wg, pcg)]
            jax.block_until_ready(placed)
            devput_cache[ck] = placed
        xs, ws, pcs = devput_cache[ck]
        out = zs
        for _ in range(n_execs):
            out = fn(xs, ws, pcs, out)
        return out

    _compiled[key] = run
    return run
